# revision 7
# baseline (speedup 1.0000x reference)
"""Fused transformer block (attention + FFN + 2 LayerNorms) on 8 TRN2 NeuronCores.

Sharding: pure data-parallel over (batch=2) x (4 query-blocks of 512 tokens).
Each core computes K/V for its batch's full 2048-token sequence, attention for
its 512 query rows over all 16 heads, then the FFN + norms for those rows.

On-chip layout is "transposed": activations live as [d_model(part), tokens(free)]
so every matmul streams with free-dim 512 (full PE rate). Scores are computed
as S^T = K_head^T-tile @ Q_head^T (k on partitions, q free), so softmax
normalization uses PE ones-column reductions and the attention@V matmul
consumes exp(S^T) directly -- no on-chip transposes anywhere (host pre/post
transposes instead).

Matmul operand precision: bf16 (fp32 PSUM accumulation). LayerNorm spine and
softmax math in fp32.
"""

import sys

sys.path.insert(0, "/opt/trn_rl_repo")

import numpy as np

B, S, D, H = 2, 2048, 1024, 16
HD = D // H
DFF = 4 * D
P = 128
SQ = 512            # query rows per core
DC = D // P         # 8 d-model chunks
FC = DFF // P       # 32 ffn chunks
NKT = S // P        # 16 k tiles
NCORES = 8
EPS = 1e-5

_CACHE = {}


def _build():
    import concourse.bass as bass
    import concourse.bacc as bacc
    import concourse.tile as tile
    from concourse import mybir

    f32 = mybir.dt.float32
    bf16 = mybir.dt.bfloat16
    AF = mybir.ActivationFunctionType
    OP = mybir.AluOpType

    nc = bacc.Bacc("TRN2", target_bir_lowering=False, debug=False,
                   num_devices=NCORES)

    # ---- DRAM I/O ----
    xt_d = nc.dram_tensor("xt", (D, S), bf16, kind="ExternalInput").ap()
    xqt_d = nc.dram_tensor("xqt", (D, SQ), bf16, kind="ExternalInput").ap()
    xqtf_d = nc.dram_tensor("xqtf", (D, SQ), f32, kind="ExternalInput").ap()
    wq_d = nc.dram_tensor("wq", (D, D), bf16, kind="ExternalInput").ap()
    wk_d = nc.dram_tensor("wk", (D, D), bf16, kind="ExternalInput").ap()
    wv_d = nc.dram_tensor("wv", (D, D), bf16, kind="ExternalInput").ap()
    w1_d = nc.dram_tensor("w1", (D, DFF), bf16, kind="ExternalInput").ap()
    w2_d = nc.dram_tensor("w2", (DFF, D), bf16, kind="ExternalInput").ap()
    bqt_d = nc.dram_tensor("bqt", (P, DC), f32, kind="ExternalInput").ap()
    bkt_d = nc.dram_tensor("bkt", (P, DC), f32, kind="ExternalInput").ap()
    bv_d = nc.dram_tensor("bv", (D,), f32, kind="ExternalInput").ap()
    b1t_d = nc.dram_tensor("b1t", (P, FC), f32, kind="ExternalInput").ap()
    b2t_d = nc.dram_tensor("b2t", (P, DC), f32, kind="ExternalInput").ap()
    g1t_d = nc.dram_tensor("g1t", (P, DC), f32, kind="ExternalInput").ap()
    be1t_d = nc.dram_tensor("be1t", (P, DC), f32, kind="ExternalInput").ap()
    g2t_d = nc.dram_tensor("g2t", (P, DC), f32, kind="ExternalInput").ap()
    be2t_d = nc.dram_tensor("be2t", (P, DC), f32, kind="ExternalInput").ap()
    out_d = nc.dram_tensor("out", (D, SQ), f32, kind="ExternalOutput").ap()

    with tile.TileContext(nc) as tc:
        with (
            tc.tile_pool(name="persist", bufs=1) as pp,
            tc.tile_pool(name="wstream", bufs=2) as wst,
            tc.tile_pool(name="work", bufs=2) as wk,
            tc.tile_pool(name="svp", bufs=4) as svp,
            tc.tile_pool(name="consts", bufs=1) as cst,
            tc.tile_pool(name="psmm", bufs=6, space="PSUM") as psmm,
            tc.tile_pool(name="psst", bufs=2, space="PSUM") as psst,
            tc.tile_pool(name="dramb", bufs=3, space="DRAM") as drp,
        ):
            def bcast(row_ap, dst_slice, nrows):
                d = drp.tile([1, SQ], f32, tag="bcd")
                nc.sync.dma_start(d, row_ap)
                nc.gpsimd.dma_start(
                    dst_slice,
                    bass.AP(tensor=d.tensor, offset=d.offset,
                            ap=[[0, nrows], [1, SQ]]))

            # ---- constants ----
            ones_bf = cst.tile([P, 1], bf16, tag="ones_bf")
            nc.vector.memset(ones_bf, 1.0)
            eps_t = cst.tile([1, 1], f32, tag="eps")
            nc.vector.memset(eps_t, EPS)
            bqt = cst.tile([P, DC], f32, tag="bqt")
            nc.sync.dma_start(bqt, bqt_d)
            bkt = cst.tile([P, DC], f32, tag="bkt")
            nc.sync.dma_start(bkt, bkt_d)
            b1t = cst.tile([P, FC], f32, tag="b1t")
            nc.sync.dma_start(b1t, b1t_d)
            b2t = cst.tile([P, DC], f32, tag="b2t")
            nc.sync.dma_start(b2t, b2t_d)
            g1t = cst.tile([P, DC], f32, tag="g1t")
            nc.sync.dma_start(g1t, g1t_d)
            be1t = cst.tile([P, DC], f32, tag="be1t")
            nc.sync.dma_start(be1t, be1t_d)
            g2t = cst.tile([P, DC], f32, tag="g2t")
            nc.sync.dma_start(g2t, g2t_d)
            be2t = cst.tile([P, DC], f32, tag="be2t")
            nc.sync.dma_start(be2t, be2t_d)
            bvb = cst.tile([P, D], f32, tag="bvb")
            nc.gpsimd.dma_start(
                bvb, bass.AP(tensor=bv_d.tensor, offset=bv_d.offset,
                             ap=[[0, P], [1, D]]))

            # ---- resident activations ----
            xT = pp.tile([P, DC, S], bf16, tag="m32a")        # 32 KB/part
            nc.sync.dma_start(xT, xt_d.rearrange("(c p) t -> p c t", p=P))
            xqT = pp.tile([P, DC, SQ], bf16, tag="s8")
            nc.sync.dma_start(xqT, xqt_d.rearrange("(c p) t -> p c t", p=P))
            xqTf = pp.tile([P, DC, SQ], f32, tag="s16a")  # residual (fp32)
            nc.sync.dma_start(xqTf, xqtf_d.rearrange("(c p) t -> p c t", p=P))
            QT = pp.tile([P, DC, SQ], bf16, tag="QT")
            KT = pp.tile([P, DC, S], bf16, tag="KT")
            V = pp.tile([P, NKT, D], bf16, tag="V")
            zT = pp.tile([P, DC, SQ], f32, tag="s16z")
            hT = pp.tile([P, DC, SQ], bf16, tag="s8")
            gT = pp.tile([P, FC, SQ], bf16, tag="m32a")
            z2T = pp.tile([P, DC, SQ], f32, tag="s16z")
            outT = pp.tile([P, DC, SQ], f32, tag="s16a")

            wq_r = wq_d.rearrange("(c p) n -> p c n", p=P)
            wk_r = wk_d.rearrange("(c p) n -> p c n", p=P)
            wv_r = wv_d.rearrange("(c p) n -> p c n", p=P)
            w1_r = w1_d.rearrange("(c p) n -> p c n", p=P)
            w2_r = w2_d.rearrange("(c p) n -> p c n", p=P)

            # ---- Q^T = Wq^T @ xq^T   [d(part), q] ----
            for half in range(2):
                wt = wst.tile([P, DC, 512], bf16, tag="w8x512")
                nc.sync.dma_start(wt, wq_r[:, :, half * 512:(half + 1) * 512])
                for g in range(4):
                    dq = half * 4 + g
                    ps = psmm.tile([P, SQ], f32, tag="mm")
                    for c in range(DC):
                        nc.tensor.matmul(ps, wt[:, c, g * P:(g + 1) * P],
                                         xqT[:, c, :],
                                         start=(c == 0), stop=(c == DC - 1))
                    nc.vector.tensor_scalar_add(QT[:, dq, :], ps,
                                                bqt[:, dq:dq + 1])

            # ---- K^T = Wk^T @ x^T   [d(part), k] over full seq ----
            for half in range(2):
                wt = wst.tile([P, DC, 512], bf16, tag="w8x512")
                nc.sync.dma_start(wt, wk_r[:, :, half * 512:(half + 1) * 512])
                for g in range(4):
                    dko = half * 4 + g
                    for tt in range(S // 512):
                        ps = psmm.tile([P, 512], f32, tag="mm")
                        for c in range(DC):
                            nc.tensor.matmul(
                                ps, wt[:, c, g * P:(g + 1) * P],
                                xT[:, c, tt * 512:(tt + 1) * 512],
                                start=(c == 0), stop=(c == DC - 1))
                        nc.vector.tensor_scalar_add(
                            KT[:, dko, tt * 512:(tt + 1) * 512], ps,
                            bkt[:, dko:dko + 1])

            # ---- V = x @ Wv + bv   [tokens(part), d_v] natural layout ----
            for half in range(2):
                wt = wst.tile([P, DC, 512], bf16, tag="w8x512")
                nc.sync.dma_start(wt, wv_r[:, :, half * 512:(half + 1) * 512])
                for tt in range(NKT):
                    ps = psmm.tile([P, 512], f32, tag="mm")
                    for c in range(DC):
                        nc.tensor.matmul(ps, xT[:, c, tt * P:(tt + 1) * P],
                                         wt[:, c, :],
                                         start=(c == 0), stop=(c == DC - 1))
                    nc.vector.tensor_add(
                        V[:, tt, half * 512:(half + 1) * 512], ps,
                        bvb[:, half * 512:(half + 1) * 512])

            # ---- attention, one head-pair (= one d-chunk) at a time ----
            for p in range(DC):
                out_ps = psmm.tile([P, SQ], f32, tag="mm")
                den_a = psmm.tile([1, SQ], f32, tag="mm")
                den_b = psmm.tile([1, SQ], f32, tag="mm")
                for kt in range(NKT):
                    first, last = kt == 0, kt == NKT - 1
                    ksl = slice(kt * P, (kt + 1) * P)
                    sa = psmm.tile([P, SQ], f32, tag="mm")
                    sb = psmm.tile([P, SQ], f32, tag="mm")
                    # S^T[k,q] for the two heads (row-groups 0-63 / 64-127)
                    nc.tensor.matmul(sa, KT[0:64, p, ksl], QT[0:64, p, :],
                                     start=True, stop=True,
                                     tile_position=(0, 0))
                    nc.tensor.matmul(sb, KT[64:128, p, ksl], QT[64:128, p, :],
                                     start=True, stop=True,
                                     tile_position=(64, 0))
                    ea = wk.tile([P, SQ], bf16, tag="ea")
                    eb = wk.tile([P, SQ], bf16, tag="eb")
                    nc.scalar.activation(ea, sa, AF.Exp, scale=0.125)
                    nc.scalar.activation(eb, sb, AF.Exp, scale=0.125)
                    # denominators (column-sum over k via ones matmul)
                    nc.tensor.matmul(den_a, ones_bf, ea, start=first, stop=last)
                    nc.tensor.matmul(den_b, ones_bf, eb, start=first, stop=last)
                    # out^T += V_head^T-packed @ exp(S^T)  (column-packed pair)
                    nc.tensor.matmul(out_ps[0:64, :],
                                     V[:, kt, p * P:p * P + 64], ea,
                                     start=first, stop=last,
                                     tile_position=(0, 0))
                    nc.tensor.matmul(out_ps[64:128, :],
                                     V[:, kt, p * P + 64:(p + 1) * P], eb,
                                     start=first, stop=last,
                                     tile_position=(0, 64))
                recip_a = svp.tile([1, SQ], f32, tag="sv")
                recip_b = svp.tile([1, SQ], f32, tag="sv")
                nc.vector.reciprocal(recip_a, den_a)
                nc.vector.reciprocal(recip_b, den_b)
                bc = wk.tile([P, SQ], f32, tag="bcast")
                bcast(recip_a, bc[0:64, :], 64)
                bcast(recip_b, bc[64:128, :], 64)
                # normalize + residual: z = attn_out + xq
                t1 = wk.tile([P, SQ], f32, tag="scratch")
                nc.vector.tensor_mul(t1, out_ps, bc)
                nc.vector.tensor_add(zT[:, p, :], t1, xqTf[:, p, :])

            # ---- layernorm helper (stats over partitions via PE) ----
            def layer_norm(src, gt, bet, dst, dst_out=None):
                sum_ps = psst.tile([1, SQ], f32, tag="st")
                ssq_ps = psst.tile([1, SQ], f32, tag="st")
                for c in range(DC):
                    zbf = wk.tile([P, SQ], bf16, tag="scratch")
                    nc.vector.tensor_copy(zbf, src[:, c, :])
                    nc.tensor.matmul(sum_ps, ones_bf, zbf,
                                     start=(c == 0), stop=(c == DC - 1))
                    zsq = wk.tile([P, SQ], bf16, tag="scratch")
                    nc.vector.tensor_mul(zsq, src[:, c, :], src[:, c, :])
                    nc.tensor.matmul(ssq_ps, ones_bf, zsq,
                                     start=(c == 0), stop=(c == DC - 1))
                mean = svp.tile([1, SQ], f32, tag="sv")
                var = svp.tile([1, SQ], f32, tag="sv")
                msq = svp.tile([1, SQ], f32, tag="sv")
                std = svp.tile([1, SQ], f32, tag="sv")
                rstd = svp.tile([1, SQ], f32, tag="sv")
                shift = svp.tile([1, SQ], f32, tag="sv")
                nc.vector.tensor_scalar_mul(mean, sum_ps, 1.0 / D)
                nc.vector.tensor_scalar_mul(var, ssq_ps, 1.0 / D)
                nc.vector.tensor_mul(msq, mean, mean)
                nc.vector.tensor_sub(var, var, msq)
                nc.scalar.activation(std, var, AF.Sqrt, bias=eps_t[0:1, 0:1])
                nc.vector.reciprocal(rstd, std)
                nc.vector.tensor_mul(shift, mean, rstd)
                nc.vector.tensor_scalar_mul(shift, shift, -1.0)
                bcA = wk.tile([P, SQ], f32, tag="bcast")
                bcast(rstd, bcA, P)
                bcB = wk.tile([P, SQ], f32, tag="bcast")
                bcast(shift, bcB, P)
                for c in range(DC):
                    tn = wk.tile([P, SQ], f32, tag="scratch")
                    nc.vector.tensor_mul(tn, src[:, c, :], bcA)
                    nc.vector.tensor_add(tn, tn, bcB)
                    nc.vector.tensor_scalar(dst[:, c, :], tn,
                                            gt[:, c:c + 1], bet[:, c:c + 1],
                                            op0=OP.mult, op1=OP.add)

            layer_norm(zT, g1t, be1t, hT)

            # ---- FFN1 + exact gelu ----
            for hc in range(FC):
                wt = wst.tile([P, DC, P], bf16, tag="w1s")
                nc.sync.dma_start(wt, w1_r[:, :, hc * P:(hc + 1) * P])
                ps = psmm.tile([P, SQ], f32, tag="mm")
                for c in range(DC):
                    nc.tensor.matmul(ps, wt[:, c, :], hT[:, c, :],
                                     start=(c == 0), stop=(c == DC - 1))
                nc.scalar.activation(gT[:, hc, :], ps, AF.Gelu,
                                     bias=b1t[:, hc:hc + 1])

            # ---- FFN2 ; z2 = 2*(ffn + b2) ----
            for oc in range(DC):
                wt = wst.tile([P, FC, P], bf16, tag="w8x512")
                nc.sync.dma_start(wt, w2_r[:, :, oc * P:(oc + 1) * P])
                ps = psmm.tile([P, SQ], f32, tag="mm")
                for hc in range(FC):
                    nc.tensor.matmul(ps, wt[:, hc, :], gT[:, hc, :],
                                     start=(hc == 0), stop=(hc == FC - 1))
                nc.vector.tensor_scalar(z2T[:, oc, :], ps, b2t[:, oc:oc + 1],
                                        2.0, op0=OP.add, op1=OP.mult)

            layer_norm(z2T, g2t, be2t, outT)

            for c in range(DC):
                nc.sync.dma_start(
                    out_d.rearrange("(c p) q -> p c q", p=P)[:, c, :],
                    outT[:, c, :])

    nc.compile()
    return nc


def _prep_inputs(inputs):
    import ml_dtypes

    f = np.float32
    bf = ml_dtypes.bfloat16
    x = np.asarray(inputs["x"], f)

    def tp(name):
        # [d_in, d_out] weight stays natural; contraction chunks on partitions
        return np.ascontiguousarray(np.asarray(inputs[name], f).astype(bf))

    shared = {
        "wq": tp("Wq"), "wk": tp("Wk"), "wv": tp("Wv"),
        "w1": tp("W1"), "w2": tp("W2"),
        "bqt": np.ascontiguousarray(np.asarray(inputs["bq"], f).reshape(DC, P).T),
        "bkt": np.ascontiguousarray(np.asarray(inputs["bk"], f).reshape(DC, P).T),
        "bv": np.ascontiguousarray(np.asarray(inputs["bv"], f)),
        "b1t": np.ascontiguousarray(np.asarray(inputs["b1"], f).reshape(FC, P).T),
        "b2t": np.ascontiguousarray(np.asarray(inputs["b2"], f).reshape(DC, P).T),
        "g1t": np.ascontiguousarray(np.asarray(inputs["g1"], f).reshape(DC, P).T),
        "be1t": np.ascontiguousarray(np.asarray(inputs["be1"], f).reshape(DC, P).T),
        "g2t": np.ascontiguousarray(np.asarray(inputs["g2"], f).reshape(DC, P).T),
        "be2t": np.ascontiguousarray(np.asarray(inputs["be2"], f).reshape(DC, P).T),
    }
    in_maps = []
    for core in range(NCORES):
        b, qb = core // 4, core % 4
        xb = x[b]                               # [S, D]
        xq = xb[qb * SQ:(qb + 1) * SQ]          # [SQ, D]
        m = dict(shared)
        m["xt"] = np.ascontiguousarray(xb.T.astype(bf))
        m["xqt"] = np.ascontiguousarray(xq.T.astype(bf))
        m["xqtf"] = np.ascontiguousarray(xq.T)
        in_maps.append(m)
    return in_maps


def kernel(**inputs):
    from concourse.bass_utils import run_bass_kernel_spmd

    if "nc" not in _CACHE:
        _CACHE["nc"] = _build()
    nc = _CACHE["nc"]
    in_maps = _prep_inputs(inputs)
    res = run_bass_kernel_spmd(nc, in_maps, core_ids=list(range(NCORES)))
    out = np.empty((B, S, D), np.float32)
    for core in range(NCORES):
        b, qb = core // 4, core % 4
        out[b, qb * SQ:(qb + 1) * SQ, :] = res.results[core]["out"].T
    return out


# revision 8
# speedup vs baseline: 83.5685x; 83.5685x over previous
"""Fused transformer block (attention + FFN + 2 LayerNorms) on 8 TRN2 NeuronCores.

Sharding: pure data-parallel over (batch=2) x (4 query-blocks of 512 tokens).
Each core computes K/V for its batch's full 2048-token sequence, attention for
its 512 query rows over all 16 heads, then the FFN + norms for those rows.

On-chip layout is "transposed": activations live as [d_model(part), tokens(free)]
so every matmul streams with free-dim 512 (full PE rate). Scores are computed
as S^T = K_head^T-tile @ Q_head^T (k on partitions, q free), so softmax
normalization uses PE ones-column reductions and the attention@V matmul
consumes exp(S^T) directly -- no on-chip transposes anywhere (host pre/post
transposes instead).

Matmul operand precision: bf16 (fp32 PSUM accumulation). LayerNorm spine and
softmax math in fp32.
"""

import sys

sys.path.insert(0, "/opt/trn_rl_repo")

import numpy as np
from contextlib import nullcontext as _nullctx

B, S, D, H = 2, 2048, 1024, 16
HD = D // H
DFF = 4 * D
P = 128
SQ = 512            # query rows per core
DC = D // P         # 8 d-model chunks
FC = DFF // P       # 32 ffn chunks
NKT = S // P        # 16 k tiles
NCORES = 8
EPS = 1e-5

_CACHE = {}


def _build(iters=1):
    import concourse.bass as bass
    import concourse.bacc as bacc
    import concourse.tile as tile
    from concourse import mybir

    f32 = mybir.dt.float32
    bf16 = mybir.dt.bfloat16
    AF = mybir.ActivationFunctionType
    OP = mybir.AluOpType

    nc = bacc.Bacc("TRN2", target_bir_lowering=False, debug=False,
                   num_devices=NCORES)

    # ---- DRAM I/O ----
    xt_d = nc.dram_tensor("xt", (D, S), bf16, kind="ExternalInput").ap()
    xqt_d = nc.dram_tensor("xqt", (D, SQ), bf16, kind="ExternalInput").ap()
    xqtf_d = nc.dram_tensor("xqtf", (D, SQ), f32, kind="ExternalInput").ap()
    wq_d = nc.dram_tensor("wq", (D, D), bf16, kind="ExternalInput").ap()
    wk_d = nc.dram_tensor("wk", (D, D), bf16, kind="ExternalInput").ap()
    wv_d = nc.dram_tensor("wv", (D, D), bf16, kind="ExternalInput").ap()
    w1_d = nc.dram_tensor("w1", (D, DFF), bf16, kind="ExternalInput").ap()
    w2_d = nc.dram_tensor("w2", (DFF, D), bf16, kind="ExternalInput").ap()
    bqt_d = nc.dram_tensor("bqt", (P, DC), f32, kind="ExternalInput").ap()
    bkt_d = nc.dram_tensor("bkt", (P, DC), f32, kind="ExternalInput").ap()
    bv_d = nc.dram_tensor("bv", (D,), f32, kind="ExternalInput").ap()
    b1t_d = nc.dram_tensor("b1t", (P, FC), f32, kind="ExternalInput").ap()
    b2t_d = nc.dram_tensor("b2t", (P, DC), f32, kind="ExternalInput").ap()
    g1t_d = nc.dram_tensor("g1t", (P, DC), f32, kind="ExternalInput").ap()
    be1t_d = nc.dram_tensor("be1t", (P, DC), f32, kind="ExternalInput").ap()
    g2t_d = nc.dram_tensor("g2t", (P, DC), f32, kind="ExternalInput").ap()
    be2t_d = nc.dram_tensor("be2t", (P, DC), f32, kind="ExternalInput").ap()
    out_d = nc.dram_tensor("out", (D, SQ), f32, kind="ExternalOutput").ap()

    with tile.TileContext(nc) as tc:
        with (
            tc.tile_pool(name="persist", bufs=1) as pp,
            tc.tile_pool(name="wstream", bufs=2) as wst,
            tc.tile_pool(name="work", bufs=2) as wk,
            tc.tile_pool(name="svp", bufs=4) as svp,
            tc.tile_pool(name="consts", bufs=1) as cst,
            tc.tile_pool(name="psmm", bufs=6, space="PSUM") as psmm,
            tc.tile_pool(name="psst", bufs=2, space="PSUM") as psst,
            tc.tile_pool(name="dramb", bufs=3, space="DRAM") as drp,
            (tc.For_i(0, iters, 1) if iters > 1 else _nullctx()),
        ):
            def bcast(row_ap, dst_slice, nrows):
                d = drp.tile([1, SQ], f32, tag="bcd")
                nc.sync.dma_start(d, row_ap)
                nc.gpsimd.dma_start(
                    dst_slice,
                    bass.AP(tensor=d.tensor, offset=d.offset,
                            ap=[[0, nrows], [1, SQ]]))

            # ---- constants ----
            ones_bf = cst.tile([P, 1], bf16, tag="ones_bf")
            nc.vector.memset(ones_bf, 1.0)
            eps_t = cst.tile([1, 1], f32, tag="eps")
            nc.vector.memset(eps_t, EPS)
            bqt = cst.tile([P, DC], f32, tag="bqt")
            nc.sync.dma_start(bqt, bqt_d)
            bkt = cst.tile([P, DC], f32, tag="bkt")
            nc.sync.dma_start(bkt, bkt_d)
            b1t = cst.tile([P, FC], f32, tag="b1t")
            nc.sync.dma_start(b1t, b1t_d)
            b2t = cst.tile([P, DC], f32, tag="b2t")
            nc.sync.dma_start(b2t, b2t_d)
            g1t = cst.tile([P, DC], f32, tag="g1t")
            nc.sync.dma_start(g1t, g1t_d)
            be1t = cst.tile([P, DC], f32, tag="be1t")
            nc.sync.dma_start(be1t, be1t_d)
            g2t = cst.tile([P, DC], f32, tag="g2t")
            nc.sync.dma_start(g2t, g2t_d)
            be2t = cst.tile([P, DC], f32, tag="be2t")
            nc.sync.dma_start(be2t, be2t_d)
            bvb = cst.tile([P, D], f32, tag="bvb")
            nc.gpsimd.dma_start(
                bvb, bass.AP(tensor=bv_d.tensor, offset=bv_d.offset,
                             ap=[[0, P], [1, D]]))

            # ---- resident activations ----
            xT = pp.tile([P, DC, S], bf16, tag="m32a")        # 32 KB/part
            nc.sync.dma_start(xT, xt_d.rearrange("(c p) t -> p c t", p=P))
            xqT = pp.tile([P, DC, SQ], bf16, tag="s8")
            nc.sync.dma_start(xqT, xqt_d.rearrange("(c p) t -> p c t", p=P))
            xqTf = pp.tile([P, DC, SQ], f32, tag="s16a")  # residual (fp32)
            nc.sync.dma_start(xqTf, xqtf_d.rearrange("(c p) t -> p c t", p=P))
            QT = pp.tile([P, DC, SQ], bf16, tag="QT")
            KT = pp.tile([P, DC, S], bf16, tag="KT")
            V = pp.tile([P, NKT, D], bf16, tag="V")
            zT = pp.tile([P, DC, SQ], f32, tag="s16z")
            hT = pp.tile([P, DC, SQ], bf16, tag="s8")
            gT = pp.tile([P, FC, SQ], bf16, tag="m32a")
            z2T = pp.tile([P, DC, SQ], f32, tag="s16z")
            outT = pp.tile([P, DC, SQ], f32, tag="s16a")

            wq_r = wq_d.rearrange("(c p) n -> p c n", p=P)
            wk_r = wk_d.rearrange("(c p) n -> p c n", p=P)
            wv_r = wv_d.rearrange("(c p) n -> p c n", p=P)
            w1_r = w1_d.rearrange("(c p) n -> p c n", p=P)
            w2_r = w2_d.rearrange("(c p) n -> p c n", p=P)

            # ---- Q^T = Wq^T @ xq^T   [d(part), q] ----
            for half in range(2):
                wt = wst.tile([P, DC, 512], bf16, tag="w8x512")
                nc.sync.dma_start(wt, wq_r[:, :, half * 512:(half + 1) * 512])
                for g in range(4):
                    dq = half * 4 + g
                    ps = psmm.tile([P, SQ], f32, tag="mm")
                    for c in range(DC):
                        nc.tensor.matmul(ps, wt[:, c, g * P:(g + 1) * P],
                                         xqT[:, c, :],
                                         start=(c == 0), stop=(c == DC - 1))
                    nc.vector.tensor_scalar_add(QT[:, dq, :], ps,
                                                bqt[:, dq:dq + 1])

            # ---- K^T = Wk^T @ x^T   [d(part), k] over full seq ----
            for half in range(2):
                wt = wst.tile([P, DC, 512], bf16, tag="w8x512")
                nc.sync.dma_start(wt, wk_r[:, :, half * 512:(half + 1) * 512])
                for g in range(4):
                    dko = half * 4 + g
                    for tt in range(S // 512):
                        ps = psmm.tile([P, 512], f32, tag="mm")
                        for c in range(DC):
                            nc.tensor.matmul(
                                ps, wt[:, c, g * P:(g + 1) * P],
                                xT[:, c, tt * 512:(tt + 1) * 512],
                                start=(c == 0), stop=(c == DC - 1))
                        nc.vector.tensor_scalar_add(
                            KT[:, dko, tt * 512:(tt + 1) * 512], ps,
                            bkt[:, dko:dko + 1])

            # ---- V = x @ Wv + bv   [tokens(part), d_v] natural layout ----
            for half in range(2):
                wt = wst.tile([P, DC, 512], bf16, tag="w8x512")
                nc.sync.dma_start(wt, wv_r[:, :, half * 512:(half + 1) * 512])
                for tt in range(NKT):
                    ps = psmm.tile([P, 512], f32, tag="mm")
                    for c in range(DC):
                        nc.tensor.matmul(ps, xT[:, c, tt * P:(tt + 1) * P],
                                         wt[:, c, :],
                                         start=(c == 0), stop=(c == DC - 1))
                    nc.vector.tensor_add(
                        V[:, tt, half * 512:(half + 1) * 512], ps,
                        bvb[:, half * 512:(half + 1) * 512])

            # ---- attention, one head-pair (= one d-chunk) at a time ----
            for p in range(DC):
                out_ps = psmm.tile([P, SQ], f32, tag="mm")
                den_a = psmm.tile([1, SQ], f32, tag="mm")
                den_b = psmm.tile([1, SQ], f32, tag="mm")
                for kt in range(NKT):
                    first, last = kt == 0, kt == NKT - 1
                    ksl = slice(kt * P, (kt + 1) * P)
                    sa = psmm.tile([P, SQ], f32, tag="mm")
                    sb = psmm.tile([P, SQ], f32, tag="mm")
                    # S^T[k,q] for the two heads (row-groups 0-63 / 64-127)
                    nc.tensor.matmul(sa, KT[0:64, p, ksl], QT[0:64, p, :],
                                     start=True, stop=True,
                                     tile_position=(0, 0))
                    nc.tensor.matmul(sb, KT[64:128, p, ksl], QT[64:128, p, :],
                                     start=True, stop=True,
                                     tile_position=(64, 0))
                    ea = wk.tile([P, SQ], bf16, tag="ea")
                    eb = wk.tile([P, SQ], bf16, tag="eb")
                    nc.scalar.activation(ea, sa, AF.Exp, scale=0.125)
                    nc.scalar.activation(eb, sb, AF.Exp, scale=0.125)
                    # denominators (column-sum over k via ones matmul)
                    nc.tensor.matmul(den_a, ones_bf, ea, start=first, stop=last)
                    nc.tensor.matmul(den_b, ones_bf, eb, start=first, stop=last)
                    # out^T += V_head^T-packed @ exp(S^T)  (column-packed pair)
                    nc.tensor.matmul(out_ps[0:64, :],
                                     V[:, kt, p * P:p * P + 64], ea,
                                     start=first, stop=last,
                                     tile_position=(0, 0))
                    nc.tensor.matmul(out_ps[64:128, :],
                                     V[:, kt, p * P + 64:(p + 1) * P], eb,
                                     start=first, stop=last,
                                     tile_position=(0, 64))
                recip_a = svp.tile([1, SQ], f32, tag="sv")
                recip_b = svp.tile([1, SQ], f32, tag="sv")
                nc.vector.reciprocal(recip_a, den_a)
                nc.vector.reciprocal(recip_b, den_b)
                bc = wk.tile([P, SQ], f32, tag="bcast")
                bcast(recip_a, bc[0:64, :], 64)
                bcast(recip_b, bc[64:128, :], 64)
                # normalize + residual: z = attn_out + xq
                t1 = wk.tile([P, SQ], f32, tag="scratch")
                nc.vector.tensor_mul(t1, out_ps, bc)
                nc.vector.tensor_add(zT[:, p, :], t1, xqTf[:, p, :])

            # ---- layernorm helper (stats over partitions via PE) ----
            def layer_norm(src, gt, bet, dst, dst_out=None):
                sum_ps = psst.tile([1, SQ], f32, tag="st")
                ssq_ps = psst.tile([1, SQ], f32, tag="st")
                for c in range(DC):
                    zbf = wk.tile([P, SQ], bf16, tag="scratch")
                    nc.vector.tensor_copy(zbf, src[:, c, :])
                    nc.tensor.matmul(sum_ps, ones_bf, zbf,
                                     start=(c == 0), stop=(c == DC - 1))
                    zsq = wk.tile([P, SQ], bf16, tag="scratch")
                    nc.vector.tensor_mul(zsq, src[:, c, :], src[:, c, :])
                    nc.tensor.matmul(ssq_ps, ones_bf, zsq,
                                     start=(c == 0), stop=(c == DC - 1))
                mean = svp.tile([1, SQ], f32, tag="sv")
                var = svp.tile([1, SQ], f32, tag="sv")
                msq = svp.tile([1, SQ], f32, tag="sv")
                std = svp.tile([1, SQ], f32, tag="sv")
                rstd = svp.tile([1, SQ], f32, tag="sv")
                shift = svp.tile([1, SQ], f32, tag="sv")
                nc.vector.tensor_scalar_mul(mean, sum_ps, 1.0 / D)
                nc.vector.tensor_scalar_mul(var, ssq_ps, 1.0 / D)
                nc.vector.tensor_mul(msq, mean, mean)
                nc.vector.tensor_sub(var, var, msq)
                nc.scalar.activation(std, var, AF.Sqrt, bias=eps_t[0:1, 0:1])
                nc.vector.reciprocal(rstd, std)
                nc.vector.tensor_mul(shift, mean, rstd)
                nc.vector.tensor_scalar_mul(shift, shift, -1.0)
                bcA = wk.tile([P, SQ], f32, tag="bcast")
                bcast(rstd, bcA, P)
                bcB = wk.tile([P, SQ], f32, tag="bcast")
                bcast(shift, bcB, P)
                for c in range(DC):
                    tn = wk.tile([P, SQ], f32, tag="scratch")
                    nc.vector.tensor_mul(tn, src[:, c, :], bcA)
                    nc.vector.tensor_add(tn, tn, bcB)
                    nc.vector.tensor_scalar(dst[:, c, :], tn,
                                            gt[:, c:c + 1], bet[:, c:c + 1],
                                            op0=OP.mult, op1=OP.add)

            layer_norm(zT, g1t, be1t, hT)

            # ---- FFN1 + exact gelu ----
            for hc in range(FC):
                wt = wst.tile([P, DC, P], bf16, tag="w1s")
                nc.sync.dma_start(wt, w1_r[:, :, hc * P:(hc + 1) * P])
                ps = psmm.tile([P, SQ], f32, tag="mm")
                for c in range(DC):
                    nc.tensor.matmul(ps, wt[:, c, :], hT[:, c, :],
                                     start=(c == 0), stop=(c == DC - 1))
                nc.scalar.activation(gT[:, hc, :], ps, AF.Gelu,
                                     bias=b1t[:, hc:hc + 1])

            # ---- FFN2 ; z2 = 2*(ffn + b2) ----
            for oc in range(DC):
                wt = wst.tile([P, FC, P], bf16, tag="w8x512")
                nc.sync.dma_start(wt, w2_r[:, :, oc * P:(oc + 1) * P])
                ps = psmm.tile([P, SQ], f32, tag="mm")
                for hc in range(FC):
                    nc.tensor.matmul(ps, wt[:, hc, :], gT[:, hc, :],
                                     start=(hc == 0), stop=(hc == FC - 1))
                nc.vector.tensor_scalar(z2T[:, oc, :], ps, b2t[:, oc:oc + 1],
                                        2.0, op0=OP.add, op1=OP.mult)

            layer_norm(z2T, g2t, be2t, outT)

            for c in range(DC):
                nc.sync.dma_start(
                    out_d.rearrange("(c p) q -> p c q", p=P)[:, c, :],
                    outT[:, c, :])

    nc.compile()
    return nc


def _prep_inputs(inputs):
    import ml_dtypes

    f = np.float32
    bf = ml_dtypes.bfloat16
    x = np.asarray(inputs["x"], f)

    def tp(name):
        # [d_in, d_out] weight stays natural; contraction chunks on partitions
        return np.ascontiguousarray(np.asarray(inputs[name], f).astype(bf))

    shared = {
        "wq": tp("Wq"), "wk": tp("Wk"), "wv": tp("Wv"),
        "w1": tp("W1"), "w2": tp("W2"),
        "bqt": np.ascontiguousarray(np.asarray(inputs["bq"], f).reshape(DC, P).T),
        "bkt": np.ascontiguousarray(np.asarray(inputs["bk"], f).reshape(DC, P).T),
        "bv": np.ascontiguousarray(np.asarray(inputs["bv"], f)),
        "b1t": np.ascontiguousarray(np.asarray(inputs["b1"], f).reshape(FC, P).T),
        "b2t": np.ascontiguousarray(np.asarray(inputs["b2"], f).reshape(DC, P).T),
        "g1t": np.ascontiguousarray(np.asarray(inputs["g1"], f).reshape(DC, P).T),
        "be1t": np.ascontiguousarray(np.asarray(inputs["be1"], f).reshape(DC, P).T),
        "g2t": np.ascontiguousarray(np.asarray(inputs["g2"], f).reshape(DC, P).T),
        "be2t": np.ascontiguousarray(np.asarray(inputs["be2"], f).reshape(DC, P).T),
    }
    in_maps = []
    for core in range(NCORES):
        b, qb = core // 4, core % 4
        xb = x[b]                               # [S, D]
        xq = xb[qb * SQ:(qb + 1) * SQ]          # [SQ, D]
        m = dict(shared)
        m["xt"] = np.ascontiguousarray(xb.T.astype(bf))
        m["xqt"] = np.ascontiguousarray(xq.T.astype(bf))
        m["xqtf"] = np.ascontiguousarray(xq.T)
        in_maps.append(m)
    return in_maps


def kernel(**inputs):
    from concourse.bass_utils import run_bass_kernel_spmd

    if "nc" not in _CACHE:
        _CACHE["nc"] = _build()
    nc = _CACHE["nc"]
    in_maps = _prep_inputs(inputs)
    res = run_bass_kernel_spmd(nc, in_maps, core_ids=list(range(NCORES)))
    out = np.empty((B, S, D), np.float32)
    for core in range(NCORES):
        b, qb = core // 4, core % 4
        out[b, qb * SQ:(qb + 1) * SQ, :] = res.results[core]["out"].T
    return out


# revision 9
# speedup vs baseline: 119.5132x; 1.4301x over previous
"""Fused transformer block (attention + FFN + 2 LayerNorms) on 8 TRN2 NeuronCores.

Sharding: pure data-parallel over (batch=2) x (4 query-blocks of 512 tokens).
Each core computes K/V for its batch's full 2048-token sequence, attention for
its 512 query rows over all 16 heads, then the FFN + norms for those rows.

On-chip layout is "transposed": activations live as [d_model(part), tokens(free)]
so every matmul streams with free-dim 512 (full PE rate). Scores are computed
as S^T = K_head^T-tile @ Q_head^T (k on partitions, q free), so softmax
normalization uses PE ones-column reductions and the attention@V matmul
consumes exp(S^T) directly -- no on-chip transposes anywhere (host pre/post
transposes instead).

Matmul operand precision: bf16 (fp32 PSUM accumulation). LayerNorm spine and
softmax math in fp32.
"""

import sys

sys.path.insert(0, "/opt/trn_rl_repo")

import numpy as np
from contextlib import nullcontext as _nullctx

B, S, D, H = 2, 2048, 1024, 16
HD = D // H
DFF = 4 * D
P = 128
SQ = 512            # query rows per core
DC = D // P         # 8 d-model chunks
FC = DFF // P       # 32 ffn chunks
NKT = S // P        # 16 k tiles
NCORES = 8
EPS = 1e-5

_CACHE = {}


def _build(iters=1):
    import concourse.bass as bass
    import concourse.bacc as bacc
    import concourse.tile as tile
    from concourse import mybir

    f32 = mybir.dt.float32
    bf16 = mybir.dt.bfloat16
    AF = mybir.ActivationFunctionType
    OP = mybir.AluOpType

    nc = bacc.Bacc("TRN2", target_bir_lowering=False, debug=False,
                   num_devices=NCORES)

    # ---- DRAM I/O ----
    xt_d = nc.dram_tensor("xt", (D, S), bf16, kind="ExternalInput").ap()
    xqt_d = nc.dram_tensor("xqt", (D, SQ), bf16, kind="ExternalInput").ap()
    xqtf_d = nc.dram_tensor("xqtf", (D, SQ), f32, kind="ExternalInput").ap()
    wq_d = nc.dram_tensor("wq", (D, D), bf16, kind="ExternalInput").ap()
    wk_d = nc.dram_tensor("wk", (D, D), bf16, kind="ExternalInput").ap()
    wv_d = nc.dram_tensor("wv", (D, D), bf16, kind="ExternalInput").ap()
    w1_d = nc.dram_tensor("w1", (D, DFF), bf16, kind="ExternalInput").ap()
    w2_d = nc.dram_tensor("w2", (DFF, D), bf16, kind="ExternalInput").ap()
    bqt_d = nc.dram_tensor("bqt", (P, DC), f32, kind="ExternalInput").ap()
    bkt_d = nc.dram_tensor("bkt", (P, DC), f32, kind="ExternalInput").ap()
    bv_d = nc.dram_tensor("bv", (D,), f32, kind="ExternalInput").ap()
    b1t_d = nc.dram_tensor("b1t", (P, FC), f32, kind="ExternalInput").ap()
    b2t_d = nc.dram_tensor("b2t", (P, DC), f32, kind="ExternalInput").ap()
    g1t_d = nc.dram_tensor("g1t", (P, DC), f32, kind="ExternalInput").ap()
    be1t_d = nc.dram_tensor("be1t", (P, DC), f32, kind="ExternalInput").ap()
    g2t_d = nc.dram_tensor("g2t", (P, DC), f32, kind="ExternalInput").ap()
    be2t_d = nc.dram_tensor("be2t", (P, DC), f32, kind="ExternalInput").ap()
    out_d = nc.dram_tensor("out", (D, SQ), f32, kind="ExternalOutput").ap()

    with tile.TileContext(nc) as tc:
        with (
            tc.tile_pool(name="persist", bufs=1) as pp,
            tc.tile_pool(name="wstream", bufs=2) as wst,
            tc.tile_pool(name="work", bufs=2) as wk,
            tc.tile_pool(name="svp", bufs=4) as svp,
            tc.tile_pool(name="consts", bufs=1) as cst,
            tc.tile_pool(name="psmm", bufs=4, space="PSUM") as psmm,
            tc.tile_pool(name="pssc", bufs=2, space="PSUM") as pssc,
            tc.tile_pool(name="dramb", bufs=3, space="DRAM") as drp,
            (tc.For_i(0, iters, 1) if iters > 1 else _nullctx()),
        ):
            def bcast(row_ap, dst_slice, nrows):
                d = drp.tile([1, SQ], f32, tag="bcd")
                nc.sync.dma_start(d, row_ap)
                nc.gpsimd.dma_start(
                    dst_slice,
                    bass.AP(tensor=d.tensor, offset=d.offset,
                            ap=[[0, nrows], [1, SQ]]))

            # ---- constants ----
            ones_bf = cst.tile([P, 1], bf16, tag="ones_bf")
            nc.vector.memset(ones_bf, 1.0)
            eps_t = cst.tile([1, 1], f32, tag="eps")
            nc.vector.memset(eps_t, EPS)
            bqt = cst.tile([P, DC], f32, tag="bqt")
            nc.sync.dma_start(bqt, bqt_d)
            bkt = cst.tile([P, DC], f32, tag="bkt")
            nc.sync.dma_start(bkt, bkt_d)
            b1t = cst.tile([P, FC], f32, tag="b1t")
            nc.sync.dma_start(b1t, b1t_d)
            b2t = cst.tile([P, DC], f32, tag="b2t")
            nc.sync.dma_start(b2t, b2t_d)
            g1t = cst.tile([P, DC], f32, tag="g1t")
            nc.sync.dma_start(g1t, g1t_d)
            be1t = cst.tile([P, DC], f32, tag="be1t")
            nc.sync.dma_start(be1t, be1t_d)
            g2t = cst.tile([P, DC], f32, tag="g2t")
            nc.sync.dma_start(g2t, g2t_d)
            be2t = cst.tile([P, DC], f32, tag="be2t")
            nc.sync.dma_start(be2t, be2t_d)
            bvb = cst.tile([P, D], f32, tag="bvb")
            nc.gpsimd.dma_start(
                bvb, bass.AP(tensor=bv_d.tensor, offset=bv_d.offset,
                             ap=[[0, P], [1, D]]))

            # ---- resident activations ----
            xT = pp.tile([P, DC, S], bf16, tag="m32a")        # 32 KB/part
            nc.sync.dma_start(xT, xt_d.rearrange("(c p) t -> p c t", p=P))
            xqT = pp.tile([P, DC, SQ], bf16, tag="s8")
            nc.sync.dma_start(xqT, xqt_d.rearrange("(c p) t -> p c t", p=P))
            xqTf = pp.tile([P, DC, SQ], f32, tag="s16a")  # residual (fp32)
            nc.sync.dma_start(xqTf, xqtf_d.rearrange("(c p) t -> p c t", p=P))
            V = pp.tile([P, NKT, D], bf16, tag="V")
            zT = pp.tile([P, DC, SQ], f32, tag="s16z")
            hT = pp.tile([P, DC, SQ], bf16, tag="s8")
            gT = pp.tile([P, FC, SQ], bf16, tag="m32a")
            z2T = pp.tile([P, DC, SQ], f32, tag="s16z")
            outT = pp.tile([P, DC, SQ], f32, tag="s16a")

            wq_r = wq_d.rearrange("(c p) n -> p c n", p=P)
            wk_r = wk_d.rearrange("(c p) n -> p c n", p=P)
            wv_r = wv_d.rearrange("(c p) n -> p c n", p=P)
            w1_r = w1_d.rearrange("(c p) n -> p c n", p=P)
            w2_r = w2_d.rearrange("(c p) n -> p c n", p=P)

            # ---- V = x @ Wv + bv   [tokens(part), d_v] natural layout ----
            for half in range(2):
                wt = wst.tile([P, DC, 512], bf16, tag="w8x512")
                nc.sync.dma_start(wt, wv_r[:, :, half * 512:(half + 1) * 512])
                for tt in range(NKT):
                    ps = psmm.tile([P, 512], f32, tag="mm")
                    for c in range(DC):
                        nc.tensor.matmul(ps, xT[:, c, tt * P:(tt + 1) * P],
                                         wt[:, c, :],
                                         start=(c == 0), stop=(c == DC - 1))
                    nc.vector.tensor_add(
                        V[:, tt, half * 512:(half + 1) * 512], ps,
                        bvb[:, half * 512:(half + 1) * 512])

            # ---- interleaved K/Q projection + attention ----
            # chunk p of K^T/Q^T is produced while pair p-1's softmax exp
            # (ACT) runs, keeping TensorE busy through the ACT-bound phase.
            def proj_group(p, grp, wkt, wqt, kt_c, qt_c):
                if grp < 4:
                    tt = grp
                    ps = psmm.tile([P, 512], f32, tag="mm")
                    for c in range(DC):
                        nc.tensor.matmul(ps, wkt[:, c, :],
                                         xT[:, c, tt * 512:(tt + 1) * 512],
                                         start=(c == 0), stop=(c == DC - 1))
                    nc.vector.tensor_scalar_add(
                        kt_c[:, tt * 512:(tt + 1) * 512], ps, bkt[:, p:p + 1])
                else:
                    ps = psmm.tile([P, SQ], f32, tag="mm")
                    for c in range(DC):
                        nc.tensor.matmul(ps, wqt[:, c, :], xqT[:, c, :],
                                         start=(c == 0), stop=(c == DC - 1))
                    nc.vector.tensor_scalar_add(qt_c, ps, bqt[:, p:p + 1])

            kt_cur = wst.tile([P, S], bf16, tag="ktc")
            qt_cur = wst.tile([P, SQ], bf16, tag="qtc")
            wkt = wst.tile([P, DC, P], bf16, tag="wks")
            nc.sync.dma_start(wkt, wk_r[:, :, 0:P])
            wqt = wst.tile([P, DC, P], bf16, tag="wqs")
            nc.sync.dma_start(wqt, wq_r[:, :, 0:P])
            for grp in range(5):
                proj_group(0, grp, wkt, wqt, kt_cur, qt_cur)

            for p in range(DC):
                if p < DC - 1:
                    kt_nxt = wst.tile([P, S], bf16, tag="ktc")
                    qt_nxt = wst.tile([P, SQ], bf16, tag="qtc")
                    wkt = wst.tile([P, DC, P], bf16, tag="wks")
                    nc.sync.dma_start(wkt, wk_r[:, :, (p + 1) * P:(p + 2) * P])
                    wqt = wst.tile([P, DC, P], bf16, tag="wqs")
                    nc.sync.dma_start(wqt, wq_r[:, :, (p + 1) * P:(p + 2) * P])
                out_ps = psmm.tile([P, SQ], f32, tag="mm")
                den_a = psmm.tile([1, SQ], f32, tag="mm")
                den_b = psmm.tile([1, SQ], f32, tag="mm")
                for w in range(8):
                    sa = pssc.tile([P, 2, SQ], f32, tag="sc2")
                    sb = pssc.tile([P, 2, SQ], f32, tag="sc2")
                    for j in range(2):
                        kt = w * 2 + j
                        ksl = slice(kt * P, (kt + 1) * P)
                        nc.tensor.matmul(sa[:, j, :], kt_cur[0:64, ksl],
                                         qt_cur[0:64, :], start=True,
                                         stop=True, tile_position=(0, 0))
                        nc.tensor.matmul(sb[:, j, :], kt_cur[64:128, ksl],
                                         qt_cur[64:128, :], start=True,
                                         stop=True, tile_position=(64, 0))
                    ea = wk.tile([P, 2, SQ], bf16, tag="ea")
                    eb = wk.tile([P, 2, SQ], bf16, tag="eb")
                    nc.scalar.activation(ea, sa, AF.Exp, scale=0.125)
                    nc.scalar.activation(eb, sb, AF.Exp, scale=0.125)
                    for j in range(2):
                        kt = w * 2 + j
                        first, last = kt == 0, kt == NKT - 1
                        nc.tensor.matmul(den_a, ones_bf, ea[:, j, :],
                                         start=first, stop=last)
                        nc.tensor.matmul(den_b, ones_bf, eb[:, j, :],
                                         start=first, stop=last)
                        nc.tensor.matmul(out_ps[0:64, :],
                                         V[:, kt, p * P:p * P + 64],
                                         ea[:, j, :], start=first, stop=last,
                                         tile_position=(0, 0))
                        nc.tensor.matmul(out_ps[64:128, :],
                                         V[:, kt, p * P + 64:(p + 1) * P],
                                         eb[:, j, :], start=first, stop=last,
                                         tile_position=(0, 64))
                    if p < DC - 1 and w in (1, 3, 5, 7):
                        proj_group(p + 1, w // 2, wkt, wqt, kt_nxt, qt_nxt)
                    if p < DC - 1 and w == 6:
                        proj_group(p + 1, 4, wkt, wqt, kt_nxt, qt_nxt)
                recip_a = svp.tile([1, SQ], f32, tag="sv")
                recip_b = svp.tile([1, SQ], f32, tag="sv")
                nc.vector.reciprocal(recip_a, den_a)
                nc.vector.reciprocal(recip_b, den_b)
                bc = wk.tile([P, SQ], f32, tag="bcast")
                bcast(recip_a, bc[0:64, :], 64)
                bcast(recip_b, bc[64:128, :], 64)
                t1 = wk.tile([P, SQ], f32, tag="scratch")
                nc.vector.tensor_mul(t1, out_ps, bc)
                nc.vector.tensor_add(zT[:, p, :], t1, xqTf[:, p, :])
                if p < DC - 1:
                    kt_cur, qt_cur = kt_nxt, qt_nxt

            # ---- layernorm helper (stats over partitions via PE) ----
            def layer_norm(src, gt, bet, dst, dst_out=None):
                sum_ps = psmm.tile([1, SQ], f32, tag="mm")
                ssq_ps = psmm.tile([1, SQ], f32, tag="mm")
                for c in range(DC):
                    zbf = wk.tile([P, SQ], bf16, tag="scratch")
                    nc.vector.tensor_copy(zbf, src[:, c, :])
                    nc.tensor.matmul(sum_ps, ones_bf, zbf,
                                     start=(c == 0), stop=(c == DC - 1))
                    zsq = wk.tile([P, SQ], bf16, tag="scratch")
                    nc.vector.tensor_mul(zsq, src[:, c, :], src[:, c, :])
                    nc.tensor.matmul(ssq_ps, ones_bf, zsq,
                                     start=(c == 0), stop=(c == DC - 1))
                mean = svp.tile([1, SQ], f32, tag="sv")
                var = svp.tile([1, SQ], f32, tag="sv")
                msq = svp.tile([1, SQ], f32, tag="sv")
                std = svp.tile([1, SQ], f32, tag="sv")
                rstd = svp.tile([1, SQ], f32, tag="sv")
                shift = svp.tile([1, SQ], f32, tag="sv")
                nc.vector.tensor_scalar_mul(mean, sum_ps, 1.0 / D)
                nc.vector.tensor_scalar_mul(var, ssq_ps, 1.0 / D)
                nc.vector.tensor_mul(msq, mean, mean)
                nc.vector.tensor_sub(var, var, msq)
                nc.scalar.activation(std, var, AF.Sqrt, bias=eps_t[0:1, 0:1])
                nc.vector.reciprocal(rstd, std)
                nc.vector.tensor_mul(shift, mean, rstd)
                nc.vector.tensor_scalar_mul(shift, shift, -1.0)
                bcA = wk.tile([P, SQ], f32, tag="bcast")
                bcast(rstd, bcA, P)
                bcB = wk.tile([P, SQ], f32, tag="bcast")
                bcast(shift, bcB, P)
                for c in range(DC):
                    tn = wk.tile([P, SQ], f32, tag="scratch")
                    nc.vector.tensor_mul(tn, src[:, c, :], bcA)
                    nc.vector.tensor_add(tn, tn, bcB)
                    nc.vector.tensor_scalar(dst[:, c, :], tn,
                                            gt[:, c:c + 1], bet[:, c:c + 1],
                                            op0=OP.mult, op1=OP.add)

            layer_norm(zT, g1t, be1t, hT)

            # ---- FFN1 + exact gelu ----
            for hc in range(FC):
                wt = wst.tile([P, DC, P], bf16, tag="w1s")
                nc.sync.dma_start(wt, w1_r[:, :, hc * P:(hc + 1) * P])
                ps = psmm.tile([P, SQ], f32, tag="mm")
                for c in range(DC):
                    nc.tensor.matmul(ps, wt[:, c, :], hT[:, c, :],
                                     start=(c == 0), stop=(c == DC - 1))
                nc.scalar.activation(gT[:, hc, :], ps, AF.Gelu,
                                     bias=b1t[:, hc:hc + 1])

            # ---- FFN2 ; z2 = 2*(ffn + b2) ----
            for oc in range(DC):
                wt = wst.tile([P, FC, P], bf16, tag="w8x512")
                nc.sync.dma_start(wt, w2_r[:, :, oc * P:(oc + 1) * P])
                ps = psmm.tile([P, SQ], f32, tag="mm")
                for hc in range(FC):
                    nc.tensor.matmul(ps, wt[:, hc, :], gT[:, hc, :],
                                     start=(hc == 0), stop=(hc == FC - 1))
                nc.vector.tensor_scalar(z2T[:, oc, :], ps, b2t[:, oc:oc + 1],
                                        2.0, op0=OP.add, op1=OP.mult)

            layer_norm(z2T, g2t, be2t, outT)

            for c in range(DC):
                nc.sync.dma_start(
                    out_d.rearrange("(c p) q -> p c q", p=P)[:, c, :],
                    outT[:, c, :])

    nc.compile()
    return nc


def _prep_inputs(inputs):
    import ml_dtypes

    f = np.float32
    bf = ml_dtypes.bfloat16
    x = np.asarray(inputs["x"], f)

    def tp(name):
        # [d_in, d_out] weight stays natural; contraction chunks on partitions
        return np.ascontiguousarray(np.asarray(inputs[name], f).astype(bf))

    shared = {
        "wq": tp("Wq"), "wk": tp("Wk"), "wv": tp("Wv"),
        "w1": tp("W1"), "w2": tp("W2"),
        "bqt": np.ascontiguousarray(np.asarray(inputs["bq"], f).reshape(DC, P).T),
        "bkt": np.ascontiguousarray(np.asarray(inputs["bk"], f).reshape(DC, P).T),
        "bv": np.ascontiguousarray(np.asarray(inputs["bv"], f)),
        "b1t": np.ascontiguousarray(np.asarray(inputs["b1"], f).reshape(FC, P).T),
        "b2t": np.ascontiguousarray(np.asarray(inputs["b2"], f).reshape(DC, P).T),
        "g1t": np.ascontiguousarray(np.asarray(inputs["g1"], f).reshape(DC, P).T),
        "be1t": np.ascontiguousarray(np.asarray(inputs["be1"], f).reshape(DC, P).T),
        "g2t": np.ascontiguousarray(np.asarray(inputs["g2"], f).reshape(DC, P).T),
        "be2t": np.ascontiguousarray(np.asarray(inputs["be2"], f).reshape(DC, P).T),
    }
    in_maps = []
    for core in range(NCORES):
        b, qb = core // 4, core % 4
        xb = x[b]                               # [S, D]
        xq = xb[qb * SQ:(qb + 1) * SQ]          # [SQ, D]
        m = dict(shared)
        m["xt"] = np.ascontiguousarray(xb.T.astype(bf))
        m["xqt"] = np.ascontiguousarray(xq.T.astype(bf))
        m["xqtf"] = np.ascontiguousarray(xq.T)
        in_maps.append(m)
    return in_maps


def kernel(**inputs):
    from concourse.bass_utils import run_bass_kernel_spmd

    if "nc" not in _CACHE:
        _CACHE["nc"] = _build()
    nc = _CACHE["nc"]
    in_maps = _prep_inputs(inputs)
    res = run_bass_kernel_spmd(nc, in_maps, core_ids=list(range(NCORES)))
    out = np.empty((B, S, D), np.float32)
    for core in range(NCORES):
        b, qb = core // 4, core % 4
        out[b, qb * SQ:(qb + 1) * SQ, :] = res.results[core]["out"].T
    return out


# revision 11
# speedup vs baseline: 122.2202x; 1.0227x over previous
"""Fused transformer block (attention + FFN + 2 LayerNorms) on 8 TRN2 NeuronCores.

Sharding: pure data-parallel over (batch=2) x (4 query-blocks of 512 tokens).
Each core computes K/V for its batch's full 2048-token sequence, attention for
its 512 query rows over all 16 heads, then the FFN + norms for those rows.

On-chip layout is "transposed": activations live as [d_model(part), tokens(free)]
so every matmul streams with free-dim 512 (full PE rate). Scores are computed
as S^T = K_head^T-tile @ Q_head^T (k on partitions, q free), so softmax
normalization uses PE ones-column reductions and the attention@V matmul
consumes exp(S^T) directly -- no on-chip transposes anywhere (host pre/post
transposes instead).

Matmul operand precision: bf16 (fp32 PSUM accumulation). LayerNorm spine and
softmax math in fp32.
"""

import sys

sys.path.insert(0, "/opt/trn_rl_repo")

import numpy as np
from contextlib import nullcontext as _nullctx

B, S, D, H = 2, 2048, 1024, 16
HD = D // H
DFF = 4 * D
P = 128
SQ = 512            # query rows per core
DC = D // P         # 8 d-model chunks
FC = DFF // P       # 32 ffn chunks
NKT = S // P        # 16 k tiles
NCORES = 8
EPS = 1e-5

_CACHE = {}


def _build(iters=1):
    import concourse.bass as bass
    import concourse.bacc as bacc
    import concourse.tile as tile
    from concourse import mybir

    f32 = mybir.dt.float32
    bf16 = mybir.dt.bfloat16
    AF = mybir.ActivationFunctionType
    OP = mybir.AluOpType

    nc = bacc.Bacc("TRN2", target_bir_lowering=False, debug=False,
                   num_devices=NCORES)

    # ---- DRAM I/O ----
    xt_d = nc.dram_tensor("xt", (D, S), bf16, kind="ExternalInput").ap()
    xqt_d = nc.dram_tensor("xqt", (D, SQ), bf16, kind="ExternalInput").ap()
    xqtf_d = nc.dram_tensor("xqtf", (D, SQ), f32, kind="ExternalInput").ap()
    wq_d = nc.dram_tensor("wq", (D, D), bf16, kind="ExternalInput").ap()
    wk_d = nc.dram_tensor("wk", (D, D), bf16, kind="ExternalInput").ap()
    wv_d = nc.dram_tensor("wv", (D, D), bf16, kind="ExternalInput").ap()
    w1_d = nc.dram_tensor("w1", (D, DFF), bf16, kind="ExternalInput").ap()
    w2_d = nc.dram_tensor("w2", (DFF, D), bf16, kind="ExternalInput").ap()
    bqt_d = nc.dram_tensor("bqt", (P, DC), f32, kind="ExternalInput").ap()
    bkt_d = nc.dram_tensor("bkt", (P, DC), f32, kind="ExternalInput").ap()
    bv_d = nc.dram_tensor("bv", (D,), f32, kind="ExternalInput").ap()
    b1t_d = nc.dram_tensor("b1t", (P, FC), f32, kind="ExternalInput").ap()
    b2t_d = nc.dram_tensor("b2t", (P, DC), f32, kind="ExternalInput").ap()
    g1t_d = nc.dram_tensor("g1t", (P, DC), f32, kind="ExternalInput").ap()
    be1t_d = nc.dram_tensor("be1t", (P, DC), f32, kind="ExternalInput").ap()
    g2t_d = nc.dram_tensor("g2t", (P, DC), f32, kind="ExternalInput").ap()
    be2t_d = nc.dram_tensor("be2t", (P, DC), f32, kind="ExternalInput").ap()
    out_d = nc.dram_tensor("out", (D, SQ), f32, kind="ExternalOutput").ap()

    with tile.TileContext(nc) as tc:
        with (
            tc.tile_pool(name="persist", bufs=1) as pp,
            tc.tile_pool(name="wstream", bufs=2) as wst,
            tc.tile_pool(name="work", bufs=2) as wk,
            tc.tile_pool(name="svp", bufs=4) as svp,
            tc.tile_pool(name="consts", bufs=1) as cst,
            tc.tile_pool(name="psmm", bufs=4, space="PSUM") as psmm,
            tc.tile_pool(name="pssc", bufs=2, space="PSUM") as pssc,
            tc.tile_pool(name="dramb", bufs=3, space="DRAM") as drp,
            (tc.For_i(0, iters, 1) if iters > 1 else _nullctx()),
        ):
            def bcast(row_ap, dst_slice, nrows):
                d = drp.tile([1, SQ], f32, tag="bcd")
                nc.sync.dma_start(d, row_ap)
                nc.gpsimd.dma_start(
                    dst_slice,
                    bass.AP(tensor=d.tensor, offset=d.offset,
                            ap=[[0, nrows], [1, SQ]]))

            # ---- constants ----
            ones_bf = cst.tile([P, 1], bf16, tag="ones_bf")
            nc.vector.memset(ones_bf, 1.0)
            ones_f = cst.tile([P, 1], f32, tag="ones_f")
            nc.vector.memset(ones_f, 1.0)
            eps_t = cst.tile([1, 1], f32, tag="eps")
            nc.vector.memset(eps_t, EPS)
            bqt = cst.tile([P, DC], f32, tag="bqt")
            nc.sync.dma_start(bqt, bqt_d)
            bkt = cst.tile([P, DC], f32, tag="bkt")
            nc.sync.dma_start(bkt, bkt_d)
            b1t = cst.tile([P, FC], f32, tag="b1t")
            nc.sync.dma_start(b1t, b1t_d)
            b2t = cst.tile([P, DC], f32, tag="b2t")
            nc.sync.dma_start(b2t, b2t_d)
            g1t = cst.tile([P, DC], f32, tag="g1t")
            nc.sync.dma_start(g1t, g1t_d)
            be1t = cst.tile([P, DC], f32, tag="be1t")
            nc.sync.dma_start(be1t, be1t_d)
            g2t = cst.tile([P, DC], f32, tag="g2t")
            nc.sync.dma_start(g2t, g2t_d)
            be2t = cst.tile([P, DC], f32, tag="be2t")
            nc.sync.dma_start(be2t, be2t_d)
            bvb = cst.tile([P, D], f32, tag="bvb")
            nc.gpsimd.dma_start(
                bvb, bass.AP(tensor=bv_d.tensor, offset=bv_d.offset,
                             ap=[[0, P], [1, D]]))

            # ---- resident activations ----
            xT = pp.tile([P, DC, S], bf16, tag="m32a")        # 32 KB/part
            nc.sync.dma_start(xT, xt_d.rearrange("(c p) t -> p c t", p=P))
            xqT = pp.tile([P, DC, SQ], bf16, tag="s8")
            nc.sync.dma_start(xqT, xqt_d.rearrange("(c p) t -> p c t", p=P))
            xqTf = pp.tile([P, DC, SQ], f32, tag="s16a")  # residual (fp32)
            nc.sync.dma_start(xqTf, xqtf_d.rearrange("(c p) t -> p c t", p=P))
            V = pp.tile([P, NKT, D], bf16, tag="V")
            zT = pp.tile([P, DC, SQ], f32, tag="s16z")
            hT = pp.tile([P, DC, SQ], bf16, tag="s8")
            gT = pp.tile([P, FC, SQ], bf16, tag="m32a")
            z2T = pp.tile([P, DC, SQ], f32, tag="s16z")
            outT = pp.tile([P, DC, SQ], f32, tag="s16a")

            wq_r = wq_d.rearrange("(c p) n -> p c n", p=P)
            wk_r = wk_d.rearrange("(c p) n -> p c n", p=P)
            wv_r = wv_d.rearrange("(c p) n -> p c n", p=P)
            w1_r = w1_d.rearrange("(c p) n -> p c n", p=P)
            w2_r = w2_d.rearrange("(c p) n -> p c n", p=P)

            # ---- V = x @ Wv + bv   [tokens(part), d_v] natural layout ----
            for half in range(2):
                wt = wst.tile([P, DC, 512], bf16, tag="w8x512")
                nc.sync.dma_start(wt, wv_r[:, :, half * 512:(half + 1) * 512])
                for tt in range(NKT):
                    ps = psmm.tile([P, 512], f32, tag="mm")
                    for c in range(DC):
                        nc.tensor.matmul(ps, xT[:, c, tt * P:(tt + 1) * P],
                                         wt[:, c, :],
                                         start=(c == 0), stop=(c == DC - 1))
                    nc.vector.tensor_add(
                        V[:, tt, half * 512:(half + 1) * 512], ps,
                        bvb[:, half * 512:(half + 1) * 512])

            # ---- interleaved K/Q projection + attention ----
            # chunk p of K^T/Q^T is produced while pair p-1's softmax exp
            # (ACT) runs, keeping TensorE busy through the ACT-bound phase.
            def proj_group(p, grp, wkt, wqt, kt_c, qt_c):
                if grp < 4:
                    tt = grp
                    ps = psmm.tile([P, 512], f32, tag="mm")
                    for c in range(DC):
                        nc.tensor.matmul(ps, wkt[:, c, :],
                                         xT[:, c, tt * 512:(tt + 1) * 512],
                                         start=(c == 0), stop=(c == DC - 1))
                    nc.vector.tensor_scalar_add(
                        kt_c[:, tt * 512:(tt + 1) * 512], ps, bkt[:, p:p + 1])
                else:
                    ps = psmm.tile([P, SQ], f32, tag="mm")
                    for c in range(DC):
                        nc.tensor.matmul(ps, wqt[:, c, :], xqT[:, c, :],
                                         start=(c == 0), stop=(c == DC - 1))
                    nc.vector.tensor_scalar_add(qt_c, ps, bqt[:, p:p + 1])

            kt_cur = wst.tile([P, S], bf16, tag="ktc")
            qt_cur = wst.tile([P, SQ], bf16, tag="qtc")
            wkt = wst.tile([P, DC, P], bf16, tag="wks")
            nc.sync.dma_start(wkt, wk_r[:, :, 0:P])
            wqt = wst.tile([P, DC, P], bf16, tag="wqs")
            nc.sync.dma_start(wqt, wq_r[:, :, 0:P])
            for grp in range(5):
                proj_group(0, grp, wkt, wqt, kt_cur, qt_cur)

            for p in range(DC):
                if p < DC - 1:
                    kt_nxt = wst.tile([P, S], bf16, tag="ktc")
                    qt_nxt = wst.tile([P, SQ], bf16, tag="qtc")
                    wkt = wst.tile([P, DC, P], bf16, tag="wks")
                    nc.sync.dma_start(wkt, wk_r[:, :, (p + 1) * P:(p + 2) * P])
                    wqt = wst.tile([P, DC, P], bf16, tag="wqs")
                    nc.sync.dma_start(wqt, wq_r[:, :, (p + 1) * P:(p + 2) * P])
                out_ps = psmm.tile([P, SQ], f32, tag="mm")
                den_ps = psmm.tile([33, SQ], f32, tag="mm")
                for w in range(8):
                    sa = pssc.tile([P, 2, SQ], f32, tag="sc2")
                    sb = pssc.tile([P, 2, SQ], f32, tag="sc2")
                    for j in range(2):
                        kt = w * 2 + j
                        ksl = slice(kt * P, (kt + 1) * P)
                        nc.tensor.matmul(sa[:, j, :], kt_cur[0:64, ksl],
                                         qt_cur[0:64, :], start=True,
                                         stop=True, tile_position=(0, 0))
                        nc.tensor.matmul(sb[:, j, :], kt_cur[64:128, ksl],
                                         qt_cur[64:128, :], start=True,
                                         stop=True, tile_position=(64, 0))
                    ea = wk.tile([P, 2, SQ], bf16, tag="ea")
                    eb = wk.tile([P, 2, SQ], bf16, tag="eb")
                    nc.scalar.activation(ea, sa, AF.Exp, scale=0.125)
                    nc.scalar.activation(eb, sb, AF.Exp, scale=0.125)
                    for j in range(2):
                        kt = w * 2 + j
                        first, last = kt == 0, kt == NKT - 1
                        nc.tensor.matmul(den_ps[0:1, :], ones_bf, ea[:, j, :],
                                         start=first, stop=last,
                                         tile_position=(0, 0))
                        nc.tensor.matmul(den_ps[32:33, :], ones_bf,
                                         eb[:, j, :], start=first, stop=last,
                                         tile_position=(0, 32))
                        nc.tensor.matmul(out_ps[0:64, :],
                                         V[:, kt, p * P:p * P + 64],
                                         ea[:, j, :], start=first, stop=last,
                                         tile_position=(0, 0))
                        nc.tensor.matmul(out_ps[64:128, :],
                                         V[:, kt, p * P + 64:(p + 1) * P],
                                         eb[:, j, :], start=first, stop=last,
                                         tile_position=(0, 64))
                    if p < DC - 1 and w in (1, 3, 5, 7):
                        proj_group(p + 1, w // 2, wkt, wqt, kt_nxt, qt_nxt)
                    if p < DC - 1 and w == 6:
                        proj_group(p + 1, 4, wkt, wqt, kt_nxt, qt_nxt)
                recip_a = svp.tile([1, SQ], f32, tag="sv")
                recip_b = svp.tile([1, SQ], f32, tag="sv")
                nc.vector.reciprocal(recip_a, den_ps[0:1, :])
                nc.vector.reciprocal(recip_b, den_ps[32:33, :])
                bc = wk.tile([P, SQ], f32, tag="bcast")
                d2 = drp.tile([2, SQ], f32, tag="bcd2")
                nc.sync.dma_start(d2[0:1, :], recip_a)
                nc.sync.dma_start(d2[1:2, :], recip_b)
                nc.gpsimd.dma_start(
                    bc,
                    bass.AP(tensor=d2.tensor, offset=d2.offset,
                            ap=[[SQ, 2], [0, 64], [1, SQ]]))
                t1 = wk.tile([P, SQ], f32, tag="scratch")
                nc.vector.tensor_mul(t1, out_ps, bc)
                nc.vector.tensor_add(zT[:, p, :], t1, xqTf[:, p, :])
                if p < DC - 1:
                    kt_cur, qt_cur = kt_nxt, qt_nxt

            # ---- layernorm helper (stats over partitions via PE) ----
            def layer_norm(src, gt, bet, dst, dst_out=None):
                sum_ps = psmm.tile([1, SQ], f32, tag="mm")
                ssq_ps = psmm.tile([1, SQ], f32, tag="mm")
                for c in range(DC):
                    nc.tensor.matmul(sum_ps, ones_f, src[:, c, :],
                                     start=(c == 0), stop=(c == DC - 1))
                    zsq = wk.tile([P, SQ], f32, tag="scratch")
                    nc.vector.tensor_mul(zsq, src[:, c, :], src[:, c, :])
                    nc.tensor.matmul(ssq_ps, ones_f, zsq,
                                     start=(c == 0), stop=(c == DC - 1))
                mean = svp.tile([1, SQ], f32, tag="sv")
                msq = svp.tile([1, SQ], f32, tag="sv")
                var = svp.tile([1, SQ], f32, tag="sv")
                std = svp.tile([1, SQ], f32, tag="sv")
                rstd = svp.tile([1, SQ], f32, tag="sv")
                shift = svp.tile([1, SQ], f32, tag="sv")
                nc.vector.tensor_scalar_mul(mean, sum_ps, 1.0 / D)
                nc.vector.tensor_mul(msq, mean, mean)
                nc.vector.scalar_tensor_tensor(var, ssq_ps, 1.0 / D, msq,
                                               op0=OP.mult, op1=OP.subtract)
                nc.scalar.activation(std, var, AF.Sqrt, bias=eps_t[0:1, 0:1])
                nc.vector.reciprocal(rstd, std)
                nc.vector.scalar_tensor_tensor(shift, mean, -1.0, rstd,
                                               op0=OP.mult, op1=OP.mult)
                bcA = wk.tile([P, SQ], f32, tag="bcast")
                bcast(rstd, bcA, P)
                bcB = wk.tile([P, SQ], f32, tag="bcast")
                bcast(shift, bcB, P)
                for c in range(DC):
                    tn = wk.tile([P, SQ], f32, tag="scratch")
                    nc.vector.tensor_mul(tn, src[:, c, :], bcA)
                    nc.vector.tensor_add(tn, tn, bcB)
                    nc.vector.tensor_scalar(dst[:, c, :], tn,
                                            gt[:, c:c + 1], bet[:, c:c + 1],
                                            op0=OP.mult, op1=OP.add)

            layer_norm(zT, g1t, be1t, hT)

            # ---- FFN1 + exact gelu ----
            for hc in range(FC):
                wt = wst.tile([P, DC, P], bf16, tag="w1s")
                nc.sync.dma_start(wt, w1_r[:, :, hc * P:(hc + 1) * P])
                ps = psmm.tile([P, SQ], f32, tag="mm")
                for c in range(DC):
                    nc.tensor.matmul(ps, wt[:, c, :], hT[:, c, :],
                                     start=(c == 0), stop=(c == DC - 1))
                nc.scalar.activation(gT[:, hc, :], ps, AF.Gelu,
                                     bias=b1t[:, hc:hc + 1])

            # ---- FFN2 ; z2 = 2*(ffn + b2) ----
            for oc in range(DC):
                wt = wst.tile([P, FC, P], bf16, tag="w8x512")
                nc.sync.dma_start(wt, w2_r[:, :, oc * P:(oc + 1) * P])
                ps = psmm.tile([P, SQ], f32, tag="mm")
                for hc in range(FC):
                    nc.tensor.matmul(ps, wt[:, hc, :], gT[:, hc, :],
                                     start=(hc == 0), stop=(hc == FC - 1))
                nc.vector.tensor_scalar(z2T[:, oc, :], ps, b2t[:, oc:oc + 1],
                                        2.0, op0=OP.add, op1=OP.mult)

            layer_norm(z2T, g2t, be2t, outT)

            for c in range(DC):
                nc.sync.dma_start(
                    out_d.rearrange("(c p) q -> p c q", p=P)[:, c, :],
                    outT[:, c, :])

    nc.compile()
    return nc


def _prep_inputs(inputs):
    import ml_dtypes

    f = np.float32
    bf = ml_dtypes.bfloat16
    x = np.asarray(inputs["x"], f)

    def tp(name):
        # [d_in, d_out] weight stays natural; contraction chunks on partitions
        return np.ascontiguousarray(np.asarray(inputs[name], f).astype(bf))

    shared = {
        "wq": tp("Wq"), "wk": tp("Wk"), "wv": tp("Wv"),
        "w1": tp("W1"), "w2": tp("W2"),
        "bqt": np.ascontiguousarray(np.asarray(inputs["bq"], f).reshape(DC, P).T),
        "bkt": np.ascontiguousarray(np.asarray(inputs["bk"], f).reshape(DC, P).T),
        "bv": np.ascontiguousarray(np.asarray(inputs["bv"], f)),
        "b1t": np.ascontiguousarray(np.asarray(inputs["b1"], f).reshape(FC, P).T),
        "b2t": np.ascontiguousarray(np.asarray(inputs["b2"], f).reshape(DC, P).T),
        "g1t": np.ascontiguousarray(np.asarray(inputs["g1"], f).reshape(DC, P).T),
        "be1t": np.ascontiguousarray(np.asarray(inputs["be1"], f).reshape(DC, P).T),
        "g2t": np.ascontiguousarray(np.asarray(inputs["g2"], f).reshape(DC, P).T),
        "be2t": np.ascontiguousarray(np.asarray(inputs["be2"], f).reshape(DC, P).T),
    }
    in_maps = []
    for core in range(NCORES):
        b, qb = core // 4, core % 4
        xb = x[b]                               # [S, D]
        xq = xb[qb * SQ:(qb + 1) * SQ]          # [SQ, D]
        m = dict(shared)
        m["xt"] = np.ascontiguousarray(xb.T.astype(bf))
        m["xqt"] = np.ascontiguousarray(xq.T.astype(bf))
        m["xqtf"] = np.ascontiguousarray(xq.T)
        in_maps.append(m)
    return in_maps


def kernel(**inputs):
    from concourse.bass_utils import run_bass_kernel_spmd

    if "nc" not in _CACHE:
        _CACHE["nc"] = _build()
    nc = _CACHE["nc"]
    in_maps = _prep_inputs(inputs)
    res = run_bass_kernel_spmd(nc, in_maps, core_ids=list(range(NCORES)))
    out = np.empty((B, S, D), np.float32)
    for core in range(NCORES):
        b, qb = core // 4, core % 4
        out[b, qb * SQ:(qb + 1) * SQ, :] = res.results[core]["out"].T
    return out


# revision 12
# speedup vs baseline: 129.2421x; 1.0575x over previous
"""Fused transformer block (attention + FFN + 2 LayerNorms) on 8 TRN2 NeuronCores.

Sharding: pure data-parallel over (batch=2) x (4 query-blocks of 512 tokens).
Each core computes K/V for its batch's full 2048-token sequence, attention for
its 512 query rows over all 16 heads, then the FFN + norms for those rows.

On-chip layout is "transposed": activations live as [d_model(part), tokens(free)]
so every matmul streams with free-dim 512 (full PE rate). Scores are computed
as S^T = K_head^T-tile @ Q_head^T (k on partitions, q free), so softmax
normalization uses PE ones-column reductions and the attention@V matmul
consumes exp(S^T) directly -- no on-chip transposes anywhere (host pre/post
transposes instead).

Matmul operand precision: bf16 (fp32 PSUM accumulation). LayerNorm spine and
softmax math in fp32.
"""

import sys

sys.path.insert(0, "/opt/trn_rl_repo")

import numpy as np
from contextlib import nullcontext as _nullctx

B, S, D, H = 2, 2048, 1024, 16
HD = D // H
DFF = 4 * D
P = 128
SQ = 512            # query rows per core
DC = D // P         # 8 d-model chunks
FC = DFF // P       # 32 ffn chunks
NKT = S // P        # 16 k tiles
NCORES = 8
EPS = 1e-5

_CACHE = {}


def _build(iters=1):
    import concourse.bass as bass
    import concourse.bacc as bacc
    import concourse.tile as tile
    from concourse import mybir

    f32 = mybir.dt.float32
    bf16 = mybir.dt.bfloat16
    AF = mybir.ActivationFunctionType
    OP = mybir.AluOpType

    nc = bacc.Bacc("TRN2", target_bir_lowering=False, debug=False,
                   num_devices=NCORES)

    # ---- DRAM I/O ----
    xt_d = nc.dram_tensor("xt", (D, S), bf16, kind="ExternalInput").ap()
    xqt_d = nc.dram_tensor("xqt", (D, SQ), bf16, kind="ExternalInput").ap()
    xqtf_d = nc.dram_tensor("xqtf", (D, SQ), f32, kind="ExternalInput").ap()
    wq_d = nc.dram_tensor("wq", (D, D), bf16, kind="ExternalInput").ap()
    wk_d = nc.dram_tensor("wk", (D, D), bf16, kind="ExternalInput").ap()
    wv_d = nc.dram_tensor("wv", (D, D), bf16, kind="ExternalInput").ap()
    w1_d = nc.dram_tensor("w1", (D, DFF), bf16, kind="ExternalInput").ap()
    w2_d = nc.dram_tensor("w2", (DFF, D), bf16, kind="ExternalInput").ap()
    bqt_d = nc.dram_tensor("bqt", (P, DC), f32, kind="ExternalInput").ap()
    bkt_d = nc.dram_tensor("bkt", (P, DC), f32, kind="ExternalInput").ap()
    bv_d = nc.dram_tensor("bv", (D,), f32, kind="ExternalInput").ap()
    b1t_d = nc.dram_tensor("b1t", (P, FC), f32, kind="ExternalInput").ap()
    b2t_d = nc.dram_tensor("b2t", (P, DC), f32, kind="ExternalInput").ap()
    g1t_d = nc.dram_tensor("g1t", (P, DC), f32, kind="ExternalInput").ap()
    be1t_d = nc.dram_tensor("be1t", (P, DC), f32, kind="ExternalInput").ap()
    g2t_d = nc.dram_tensor("g2t", (P, DC), f32, kind="ExternalInput").ap()
    be2t_d = nc.dram_tensor("be2t", (P, DC), f32, kind="ExternalInput").ap()
    out_d = nc.dram_tensor("out", (D, SQ), f32, kind="ExternalOutput").ap()

    with tile.TileContext(nc) as tc:
        with (
            tc.tile_pool(name="persist", bufs=1) as pp,
            tc.tile_pool(name="wstream", bufs=3) as wst,
            tc.tile_pool(name="work", bufs=3) as wk,
            tc.tile_pool(name="svp", bufs=4) as svp,
            tc.tile_pool(name="consts", bufs=1) as cst,
            tc.tile_pool(name="psmm", bufs=4, space="PSUM") as psmm,
            tc.tile_pool(name="pssc", bufs=2, space="PSUM") as pssc,
            tc.tile_pool(name="dramb", bufs=3, space="DRAM") as drp,
            (tc.For_i(0, iters, 1) if iters > 1 else _nullctx()),
        ):
            def bcast(row_ap, dst_slice, nrows):
                d = drp.tile([1, SQ], f32, tag="bcd")
                nc.sync.dma_start(d, row_ap)
                nc.gpsimd.dma_start(
                    dst_slice,
                    bass.AP(tensor=d.tensor, offset=d.offset,
                            ap=[[0, nrows], [1, SQ]]))

            # ---- constants ----
            ones_bf = cst.tile([P, 1], bf16, tag="ones_bf")
            nc.vector.memset(ones_bf, 1.0)
            ones_f = cst.tile([P, 1], f32, tag="ones_f")
            nc.vector.memset(ones_f, 1.0)
            eps_t = cst.tile([1, 1], f32, tag="eps")
            nc.vector.memset(eps_t, EPS)
            bqt = cst.tile([P, DC], f32, tag="bqt")
            nc.sync.dma_start(bqt, bqt_d)
            bkt = cst.tile([P, DC], f32, tag="bkt")
            nc.sync.dma_start(bkt, bkt_d)
            b1t = cst.tile([P, FC], f32, tag="b1t")
            nc.sync.dma_start(b1t, b1t_d)
            b2t = cst.tile([P, DC], f32, tag="b2t")
            nc.sync.dma_start(b2t, b2t_d)
            g1t = cst.tile([P, DC], f32, tag="g1t")
            nc.sync.dma_start(g1t, g1t_d)
            be1t = cst.tile([P, DC], f32, tag="be1t")
            nc.sync.dma_start(be1t, be1t_d)
            g2t = cst.tile([P, DC], f32, tag="g2t")
            nc.sync.dma_start(g2t, g2t_d)
            be2t = cst.tile([P, DC], f32, tag="be2t")
            nc.sync.dma_start(be2t, be2t_d)
            bvb = cst.tile([P, D], f32, tag="bvb")
            nc.gpsimd.dma_start(
                bvb, bass.AP(tensor=bv_d.tensor, offset=bv_d.offset,
                             ap=[[0, P], [1, D]]))

            # ---- resident activations ----
            xT = pp.tile([P, DC, S], bf16, tag="m32a")        # 32 KB/part
            nc.sync.dma_start(xT, xt_d.rearrange("(c p) t -> p c t", p=P))
            xqT = pp.tile([P, DC, SQ], bf16, tag="s8")
            nc.sync.dma_start(xqT, xqt_d.rearrange("(c p) t -> p c t", p=P))
            xqTf = pp.tile([P, DC, SQ], f32, tag="s16a")  # residual (fp32)
            nc.sync.dma_start(xqTf, xqtf_d.rearrange("(c p) t -> p c t", p=P))
            V = pp.tile([P, NKT, D], bf16, tag="V")
            zT = pp.tile([P, DC, SQ], f32, tag="s16z")
            hT = pp.tile([P, DC, SQ], bf16, tag="s8")
            gT = pp.tile([P, FC, SQ], bf16, tag="m32a")
            z2T = pp.tile([P, DC, SQ], f32, tag="s16z")
            outT = pp.tile([P, DC, SQ], f32, tag="s16a")

            wq_r = wq_d.rearrange("(c p) n -> p c n", p=P)
            wk_r = wk_d.rearrange("(c p) n -> p c n", p=P)
            wv_r = wv_d.rearrange("(c p) n -> p c n", p=P)
            w1_r = w1_d.rearrange("(c p) n -> p c n", p=P)
            w2_r = w2_d.rearrange("(c p) n -> p c n", p=P)

            # ---- V = x @ Wv + bv   [tokens(part), d_v] natural layout ----
            for half in range(2):
                wt = wst.tile([P, DC, 512], bf16, tag="w8x512")
                nc.sync.dma_start(wt, wv_r[:, :, half * 512:(half + 1) * 512])
                for tt in range(NKT):
                    ps = psmm.tile([P, 512], f32, tag="mm")
                    for c in range(DC):
                        nc.tensor.matmul(ps, xT[:, c, tt * P:(tt + 1) * P],
                                         wt[:, c, :],
                                         start=(c == 0), stop=(c == DC - 1))
                    nc.vector.tensor_add(
                        V[:, tt, half * 512:(half + 1) * 512], ps,
                        bvb[:, half * 512:(half + 1) * 512])

            # ---- interleaved K/Q projection + attention ----
            # chunk p of K^T/Q^T is produced while pair p-1's softmax exp
            # (ACT) runs, keeping TensorE busy through the ACT-bound phase.
            def proj_group(p, grp, wkt, wqt, kt_c, qt_c):
                if grp < 4:
                    tt = grp
                    ps = psmm.tile([P, 512], f32, tag="mm")
                    for c in range(DC):
                        nc.tensor.matmul(ps, wkt[:, c, :],
                                         xT[:, c, tt * 512:(tt + 1) * 512],
                                         start=(c == 0), stop=(c == DC - 1))
                    nc.vector.tensor_scalar_add(
                        kt_c[:, tt * 512:(tt + 1) * 512], ps, bkt[:, p:p + 1])
                else:
                    ps = psmm.tile([P, SQ], f32, tag="mm")
                    for c in range(DC):
                        nc.tensor.matmul(ps, wqt[:, c, :], xqT[:, c, :],
                                         start=(c == 0), stop=(c == DC - 1))
                    nc.vector.tensor_scalar_add(qt_c, ps, bqt[:, p:p + 1])

            kt_cur = wst.tile([P, S], bf16, tag="ktc")
            qt_cur = wst.tile([P, SQ], bf16, tag="qtc")
            wkt = wst.tile([P, DC, P], bf16, tag="wks")
            nc.sync.dma_start(wkt, wk_r[:, :, 0:P])
            wqt = wst.tile([P, DC, P], bf16, tag="wqs")
            nc.sync.dma_start(wqt, wq_r[:, :, 0:P])
            for grp in range(5):
                proj_group(0, grp, wkt, wqt, kt_cur, qt_cur)

            for p in range(DC):
                if p < DC - 1:
                    kt_nxt = wst.tile([P, S], bf16, tag="ktc")
                    qt_nxt = wst.tile([P, SQ], bf16, tag="qtc")
                    wkt = wst.tile([P, DC, P], bf16, tag="wks")
                    nc.sync.dma_start(wkt, wk_r[:, :, (p + 1) * P:(p + 2) * P])
                    wqt = wst.tile([P, DC, P], bf16, tag="wqs")
                    nc.sync.dma_start(wqt, wq_r[:, :, (p + 1) * P:(p + 2) * P])
                out_ps = psmm.tile([P, SQ], f32, tag="mm")
                den_ps = psmm.tile([33, SQ], f32, tag="mm")
                for w in range(8):
                    sa = pssc.tile([P, 2, SQ], f32, tag="sc2")
                    sb = pssc.tile([P, 2, SQ], f32, tag="sc2")
                    for j in range(2):
                        kt = w * 2 + j
                        ksl = slice(kt * P, (kt + 1) * P)
                        nc.tensor.matmul(sa[:, j, :], kt_cur[0:64, ksl],
                                         qt_cur[0:64, :], start=True,
                                         stop=True, tile_position=(0, 0))
                        nc.tensor.matmul(sb[:, j, :], kt_cur[64:128, ksl],
                                         qt_cur[64:128, :], start=True,
                                         stop=True, tile_position=(64, 0))
                    ea = wk.tile([P, 2, SQ], bf16, tag="ea")
                    eb = wk.tile([P, 2, SQ], bf16, tag="eb")
                    nc.scalar.activation(ea, sa, AF.Exp, scale=0.125)
                    nc.scalar.activation(eb, sb, AF.Exp, scale=0.125)
                    for j in range(2):
                        kt = w * 2 + j
                        first, last = kt == 0, kt == NKT - 1
                        nc.tensor.matmul(den_ps[0:1, :], ones_bf, ea[:, j, :],
                                         start=first, stop=last,
                                         tile_position=(0, 0))
                        nc.tensor.matmul(den_ps[32:33, :], ones_bf,
                                         eb[:, j, :], start=first, stop=last,
                                         tile_position=(0, 32))
                        nc.tensor.matmul(out_ps[0:64, :],
                                         V[:, kt, p * P:p * P + 64],
                                         ea[:, j, :], start=first, stop=last,
                                         tile_position=(0, 0))
                        nc.tensor.matmul(out_ps[64:128, :],
                                         V[:, kt, p * P + 64:(p + 1) * P],
                                         eb[:, j, :], start=first, stop=last,
                                         tile_position=(0, 64))
                    if p < DC - 1 and w in (1, 3, 5, 7):
                        proj_group(p + 1, w // 2, wkt, wqt, kt_nxt, qt_nxt)
                    if p < DC - 1 and w == 6:
                        proj_group(p + 1, 4, wkt, wqt, kt_nxt, qt_nxt)
                recip_a = svp.tile([1, SQ], f32, tag="sv")
                recip_b = svp.tile([1, SQ], f32, tag="sv")
                nc.vector.reciprocal(recip_a, den_ps[0:1, :])
                nc.vector.reciprocal(recip_b, den_ps[32:33, :])
                bc = wk.tile([P, SQ], f32, tag="bcast")
                d2 = drp.tile([2, SQ], f32, tag="bcd2")
                nc.sync.dma_start(d2[0:1, :], recip_a)
                nc.sync.dma_start(d2[1:2, :], recip_b)
                nc.gpsimd.dma_start(
                    bc,
                    bass.AP(tensor=d2.tensor, offset=d2.offset,
                            ap=[[SQ, 2], [0, 64], [1, SQ]]))
                t1 = wk.tile([P, SQ], f32, tag="scratch")
                nc.vector.tensor_mul(t1, out_ps, bc)
                nc.vector.tensor_add(zT[:, p, :], t1, xqTf[:, p, :])
                if p < DC - 1:
                    kt_cur, qt_cur = kt_nxt, qt_nxt

            # ---- layernorm helper (stats over partitions via PE) ----
            def layer_norm(src, gt, bet, dst, dst_out=None):
                sum_ps = psmm.tile([1, SQ], f32, tag="mm")
                ssq_ps = psmm.tile([1, SQ], f32, tag="mm")
                for c in range(DC):
                    nc.tensor.matmul(sum_ps, ones_f, src[:, c, :],
                                     start=(c == 0), stop=(c == DC - 1))
                    zsq = wk.tile([P, SQ], f32, tag="scratch")
                    nc.vector.tensor_mul(zsq, src[:, c, :], src[:, c, :])
                    nc.tensor.matmul(ssq_ps, ones_f, zsq,
                                     start=(c == 0), stop=(c == DC - 1))
                mean = svp.tile([1, SQ], f32, tag="sv")
                msq = svp.tile([1, SQ], f32, tag="sv")
                var = svp.tile([1, SQ], f32, tag="sv")
                std = svp.tile([1, SQ], f32, tag="sv")
                rstd = svp.tile([1, SQ], f32, tag="sv")
                shift = svp.tile([1, SQ], f32, tag="sv")
                nc.vector.tensor_scalar_mul(mean, sum_ps, 1.0 / D)
                nc.vector.tensor_mul(msq, mean, mean)
                nc.vector.scalar_tensor_tensor(var, ssq_ps, 1.0 / D, msq,
                                               op0=OP.mult, op1=OP.subtract)
                nc.scalar.activation(std, var, AF.Sqrt, bias=eps_t[0:1, 0:1])
                nc.vector.reciprocal(rstd, std)
                nc.vector.scalar_tensor_tensor(shift, mean, -1.0, rstd,
                                               op0=OP.mult, op1=OP.mult)
                bcA = wk.tile([P, SQ], f32, tag="bcast")
                bcast(rstd, bcA, P)
                bcB = wk.tile([P, SQ], f32, tag="bcast")
                bcast(shift, bcB, P)
                for c in range(DC):
                    tn = wk.tile([P, SQ], f32, tag="scratch")
                    nc.vector.tensor_mul(tn, src[:, c, :], bcA)
                    nc.vector.tensor_add(tn, tn, bcB)
                    nc.vector.tensor_scalar(dst[:, c, :], tn,
                                            gt[:, c:c + 1], bet[:, c:c + 1],
                                            op0=OP.mult, op1=OP.add)

            layer_norm(zT, g1t, be1t, hT)

            # ---- FFN1 + exact gelu ----
            for hc in range(FC):
                wt = wst.tile([P, DC, P], bf16, tag="w1s")
                nc.sync.dma_start(wt, w1_r[:, :, hc * P:(hc + 1) * P])
                ps = psmm.tile([P, SQ], f32, tag="mm")
                for c in range(DC):
                    nc.tensor.matmul(ps, wt[:, c, :], hT[:, c, :],
                                     start=(c == 0), stop=(c == DC - 1))
                nc.scalar.activation(gT[:, hc, :], ps, AF.Gelu,
                                     bias=b1t[:, hc:hc + 1])

            # ---- FFN2 ; z2 = 2*(ffn + b2) ----
            for oc in range(DC):
                wt = wst.tile([P, FC, P], bf16, tag="w8x512")
                nc.sync.dma_start(wt, w2_r[:, :, oc * P:(oc + 1) * P])
                ps = psmm.tile([P, SQ], f32, tag="mm")
                for hc in range(FC):
                    nc.tensor.matmul(ps, wt[:, hc, :], gT[:, hc, :],
                                     start=(hc == 0), stop=(hc == FC - 1))
                nc.vector.tensor_scalar(z2T[:, oc, :], ps, b2t[:, oc:oc + 1],
                                        2.0, op0=OP.add, op1=OP.mult)

            layer_norm(z2T, g2t, be2t, outT)

            for c in range(DC):
                nc.sync.dma_start(
                    out_d.rearrange("(c p) q -> p c q", p=P)[:, c, :],
                    outT[:, c, :])

    nc.compile()
    return nc


def _prep_inputs(inputs):
    import ml_dtypes

    f = np.float32
    bf = ml_dtypes.bfloat16
    x = np.asarray(inputs["x"], f)

    def tp(name):
        # [d_in, d_out] weight stays natural; contraction chunks on partitions
        return np.ascontiguousarray(np.asarray(inputs[name], f).astype(bf))

    shared = {
        "wq": tp("Wq"), "wk": tp("Wk"), "wv": tp("Wv"),
        "w1": tp("W1"), "w2": tp("W2"),
        "bqt": np.ascontiguousarray(np.asarray(inputs["bq"], f).reshape(DC, P).T),
        "bkt": np.ascontiguousarray(np.asarray(inputs["bk"], f).reshape(DC, P).T),
        "bv": np.ascontiguousarray(np.asarray(inputs["bv"], f)),
        "b1t": np.ascontiguousarray(np.asarray(inputs["b1"], f).reshape(FC, P).T),
        "b2t": np.ascontiguousarray(np.asarray(inputs["b2"], f).reshape(DC, P).T),
        "g1t": np.ascontiguousarray(np.asarray(inputs["g1"], f).reshape(DC, P).T),
        "be1t": np.ascontiguousarray(np.asarray(inputs["be1"], f).reshape(DC, P).T),
        "g2t": np.ascontiguousarray(np.asarray(inputs["g2"], f).reshape(DC, P).T),
        "be2t": np.ascontiguousarray(np.asarray(inputs["be2"], f).reshape(DC, P).T),
    }
    in_maps = []
    for core in range(NCORES):
        b, qb = core // 4, core % 4
        xb = x[b]                               # [S, D]
        xq = xb[qb * SQ:(qb + 1) * SQ]          # [SQ, D]
        m = dict(shared)
        m["xt"] = np.ascontiguousarray(xb.T.astype(bf))
        m["xqt"] = np.ascontiguousarray(xq.T.astype(bf))
        m["xqtf"] = np.ascontiguousarray(xq.T)
        in_maps.append(m)
    return in_maps


def kernel(**inputs):
    from concourse.bass_utils import run_bass_kernel_spmd

    if "nc" not in _CACHE:
        _CACHE["nc"] = _build()
    nc = _CACHE["nc"]
    in_maps = _prep_inputs(inputs)
    res = run_bass_kernel_spmd(nc, in_maps, core_ids=list(range(NCORES)))
    out = np.empty((B, S, D), np.float32)
    for core in range(NCORES):
        b, qb = core // 4, core % 4
        out[b, qb * SQ:(qb + 1) * SQ, :] = res.results[core]["out"].T
    return out
